# revision 43
# baseline (speedup 1.0000x reference)
# Trainium2 Bass kernel for AtomTypeGNN message passing.
#
#   adj_exp[m,k] = sum_n dist_adj[m,n] * dist_exp[m,n,k]          (streams dist_exp)
#   feat[m,o]    = sum_{f,h} adj_exp[m,f] * w[f,h,o] * emb[m,h]
#   out          = softplus(feat) + b
#
# Pure data parallel over atoms: 8 NeuronCores, 256 atoms each, no collectives.
#
# Final design, ~165us HW exec (baseline bf16 PE-only: ~290us):
#  * dist_exp streams as fp8 e3m4 (HW-measured l2 ~1.5e-2 vs the 2e-2 gate)
#    -- HBM bytes halve to ~33.6MB/core.
#  * The PE moving port (128 elem/cycle) cannot stream everything, so atom
#    groups split between consumers:
#    - PE groups: per atom, 8 accumulating matmuls with a [128,2] stationary
#      (two adjacent adj chunk-columns) and [128,128] moving (two chunks).
#      The [2,128] psum block accumulates even-chunk sums on row 0 cols 0:64
#      and odd-chunk sums on row 1 cols 64:128 (diagonal); off-diagonal is
#      junk. One [2,512] psum bank per 4-atom group, single start/stop.
#    - DVE groups: ScalarE casts the fp8 tile to bf16 (fp8-source DVE ops
#      run ~3-5x slow, bf16 runs near spec); DVE does ONE tensor_tensor per
#      group (adj broadcast over k via a stride-0 AP) and an in-place
#      pairwise add tree down to 2 partial sums per atom; a [128,2]
#      all-ones stationary matmul turns those into a [2,512] psum block
#      whose rows are identical, so cols 0:64/64:128 per atom mimic the PE
#      diag layout. Ones-matmuls are deferred several groups so the
#      in-order PE queue never waits on DVE.
#  * Uniform ScalarE evac [2,512]->fp16 stage2; per block two SBUF->SBUF
#    DMAs transpose the two half-sums onto atom partitions; one DVE add
#    forms adj_exp. No DRAM bounce.
#  * G_f = emb @ w[f]: 16 [128x512] chunk matmuls per block (embT
#    stationary), evacuated to bf16 gsb by DVE tensor_copy.
#  * step 2 per block: DVE STT chain over f<NCH + ScalarE scaled copies for
#    the rest, folded by a DVE pairwise tree; ScalarE-heavy mid-stream,
#    DVE-heavy in the tail. softplus = relu(x)+ln(1+exp(-min(|x|,87))).
#
# Queues: exp stream owns nc.sync (SP HWDGE); consts load on nc.scalar
# (Activation HWDGE); transpose + output DMAs ride gpsimd (SWDGE).

import numpy as np
import ml_dtypes

N = 2048
K = 64
H = 128
OUT = 128
N_CORES = 8
M = N // N_CORES  # 256 atoms per core

_BF = ml_dtypes.bfloat16
_E3 = ml_dtypes.float8_e3m4

_CACHE = {}

# tuning knobs
DVE_GROUPS = (2, 6, 10, 14, 18, 22)  # groups handled by DVE
DEFER = 12       # groups between a DVE group and its ones-matmul
G_START = 8      # first group index carrying a G chunk
NCH_STEADY = 40  # f-chain length on DVE for non-tail blocks
NCH_TAIL = 32    # f-chain length on DVE for the last block


def _ensure_path():
    import sys

    for p in ("/opt/trn_rl_repo",):
        if p not in sys.path:
            sys.path.insert(0, p)


def _build():
    _ensure_path()
    import concourse.bass as bass  # noqa: F401
    import concourse.tile as tile
    from concourse import bacc, mybir

    if not getattr(bacc, "_act_tbl_patched", False):
        _orig_gat = bacc.get_activation_tables

        def _gat(arch):
            t = _orig_gat(arch)
            key = "natural_log_exp_and_others"
            if key not in t:
                return t
            shared = t[key]
            out = {}
            for k, v in t.items():
                if k == key:
                    out[k] = v
                else:
                    out[k] = v - shared
            return out

        bacc.get_activation_tables = _gat
        bacc._act_tbl_patched = True


    f32 = mybir.dt.float32
    f16 = mybir.dt.float16
    bf16 = mybir.dt.bfloat16
    fp8 = mybir.dt.float8e3

    nc = bacc.Bacc(
        "TRN2",
        target_bir_lowering=False,
        debug=False,
        num_devices=N_CORES,
    )

    # [t, p, mq]: atom group t = atoms 4t..4t+3, partition p, mq = 1024*mm + q,
    # q = 64*c + k, n = 16p + c.  4 KiB contiguous per partition per group.
    exp_d = nc.declare_dram_parameter("exp", [M // 4, 128, 4096], fp8, isOutput=False)
    # adjA[j, 16m + c] = dist_adj[m, 16j + c]
    adjA_d = nc.declare_dram_parameter("adjA", [128, 16 * M], bf16, isOutput=False)
    embT_d = nc.declare_dram_parameter("embT", [H, M], bf16, isOutput=False)
    # w2[h, 128f + o] = bilinear_w[f, h, o]
    w_d = nc.declare_dram_parameter("w", [H, K * OUT], bf16, isOutput=False)
    bias_d = nc.declare_dram_parameter("bias", [128, OUT], f32, isOutput=False)
    ones_d = nc.declare_dram_parameter("ones", [128, 2], bf16, isOutput=False)
    out_d = nc.declare_dram_parameter("out", [M, OUT], f32, isOutput=True)

    NB = M // 128  # blocks of 128 atoms
    NG = 32        # groups per block

    with tile.TileContext(nc) as tc, nc.allow_low_precision(
        reason="fp16/bf16 adj_exp partials; fp8 quantization error dominates"
    ):
        with (
            tc.tile_pool(name="const", bufs=1) as constp,
            tc.tile_pool(name="exp", bufs=6) as expp,
            tc.tile_pool(name="etb", bufs=3) as etbp,
            tc.tile_pool(name="prod", bufs=2) as prodp,
            tc.tile_pool(name="a4", bufs=6) as a4p,
            tc.tile_pool(name="ps1", bufs=5, space="PSUM") as ps1p,
            tc.tile_pool(name="pso", bufs=1, space="PSUM") as psop,
            tc.tile_pool(name="ps2", bufs=2, space="PSUM") as ps2p,
            tc.tile_pool(name="stage", bufs=2) as stagep,
            tc.tile_pool(name="aexp2", bufs=2) as aexp2p,
            tc.tile_pool(name="aexp", bufs=2) as aexpp,
                        tc.tile_pool(name="gsb", bufs=2) as gsbp,
            tc.tile_pool(name="acc", bufs=3) as accp,
            tc.tile_pool(name="ybuf", bufs=3) as ybufp,
            tc.tile_pool(name="outp", bufs=4) as outp,
        ):
            adjA = constp.tile([128, 16 * M], bf16, tag="adjA")
            nc.scalar.dma_start(adjA[:], adjA_d[:, :])
            wsb = constp.tile([128, K * OUT], bf16, tag="wsb")
            nc.scalar.dma_start(wsb[:], w_d[:, :])
            embT = constp.tile([128, M], bf16, tag="embT")
            nc.scalar.dma_start(embT[:], embT_d[:, :])
            biassb = constp.tile([128, OUT], f32, tag="bias")
            nc.scalar.dma_start(biassb[:], bias_d[:, :])
            ones2 = constp.tile([128, 2], bf16, tag="ones")
            nc.scalar.dma_start(ones2[:], ones_d[:, :])

            # ---------------- step 2 emission helper ----------------
            def step2_pieces(blk, aexp, gsb, is_tail):
                # feat = sum_f aexp[:,f] * G_f  done in 4 f-quarters: one
                # broadcast tensor_tensor (gsb x aexp bcast over o) into a
                # fp16 ybuf, an in-place fp16 pairwise tree over the 16 f's,
                # then accumulate the quarter partial into a f32 acc. All on
                # DVE in few big ops; no per-f chain, no ScalarE copies.
                state = {}

                ybufs = {}

                def q_mult(q):
                    def emit():
                        # ScalarE materializes the aexp broadcast so the DVE
                        # multiply gets all-packed 2-byte operands (2x mode)
                        ar = ybufp.tile([128, 16 * OUT], f16, tag="ybuf")
                        ar_in = (
                            aexp[:, 16 * q : 16 * (q + 1)]
                            .unsqueeze(2)
                            .broadcast_to([128, 16, OUT])
                        )
                        if is_tail:
                            nc.scalar.activation(
                                ar[:].rearrange("p (f o) -> p f o", o=OUT),
                                ar_in,
                                mybir.ActivationFunctionType.Copy,
                            )
                        else:
                            nc.vector.tensor_copy(
                                ar[:].rearrange("p (f o) -> p f o", o=OUT),
                                ar_in,
                            )
                        yq = ybufp.tile([128, 16 * OUT], f16, tag="ybuf")
                        ybufs[q] = yq
                        nc.vector.tensor_tensor(
                            yq[:],
                            gsb[:, 2048 * q : 2048 * (q + 1)],
                            ar[:],
                            mybir.AluOpType.mult,
                        )
                    return emit

                def q_tree(qa, qb):
                    # interleave two quarters' pairwise trees so the DVE
                    # pipeline always has an independent op in flight
                    def emit():
                        w = 16
                        while w > 1:
                            h = w // 2
                            for q in (qa, qb):
                                yq = ybufs[q]
                                nc.vector.tensor_add(
                                    yq[:, 0 : h * OUT],
                                    yq[:, 0 : h * OUT],
                                    yq[:, h * OUT : 2 * h * OUT],
                                )
                            w = h
                        for q in (qa, qb):
                            yq = ybufs.pop(q)
                            acc = accp.tile([128, OUT], f32, tag="acc")
                            if state.get("acc") is None:
                                nc.vector.tensor_copy(acc[:], yq[:, 0:OUT])
                            else:
                                nc.vector.tensor_add(
                                    acc[:], state["acc"][:], yq[:, 0:OUT]
                                )
                            state["acc"] = acc
                    return emit

                def finish():
                    acc = state["acc"]
                    t_cl = outp.tile([128, OUT], f32, tag="outp")
                    nc.vector.tensor_scalar_min(t_cl[:], acc[:], 30.0)
                    t_exp = outp.tile([128, OUT], f32, tag="outp")
                    nc.scalar.activation(
                        t_exp[:], t_cl[:], mybir.ActivationFunctionType.Exp
                    )
                    t_ln = outp.tile([128, OUT], f32, tag="outp")
                    nc.scalar.activation(
                        t_ln[:], t_exp[:], mybir.ActivationFunctionType.Ln,
                        bias=1.0,
                    )
                    t_rel = outp.tile([128, OUT], f32, tag="outp")
                    nc.vector.tensor_scalar(
                        t_rel[:], acc[:], -30.0, 0.0,
                        mybir.AluOpType.add, mybir.AluOpType.max,
                    )
                    t_s = outp.tile([128, OUT], f32, tag="outp")
                    nc.vector.tensor_add(t_s[:], t_ln[:], t_rel[:])
                    ot = outp.tile([128, OUT], f32, tag="outp")
                    nc.vector.tensor_add(ot[:], t_s[:], biassb[:])
                    odq = nc.sync if is_tail else nc.gpsimd
                    odq.dma_start(
                        out_d[128 * blk : 128 * (blk + 1), :], ot[:]
                    )

                pieces = [
                    (0, q_mult(0)),
                    (2, q_mult(1)),
                    (4, q_tree(0, 1)),
                    (6, q_mult(2)),
                    (8, q_mult(3)),
                    (10, q_tree(2, 3)),
                    (13, finish),
                ]
                return pieces

            pending_step2 = []

            for blk in range(NB):
                gsb = gsbp.tile([128, K * OUT], bf16, tag="gsb")
                stage2 = stagep.tile([2, 128 * 128], f16, tag="stage")
                deferred = []
                # prefetch the last DVE group's tile via a casting DMA
                # (SWDGE converts fp8->bf16; issued ~40us ahead of use so
                # the slow software queue is off the critical path)
                pf_g = DVE_GROUPS[-1]
                etb_pf = etbp.tile([128, 4096], bf16, tag="etb")
                nc.gpsimd.dma_start(etb_pf[:], exp_d[blk * NG + pf_g])

                for g in range(NG):
                    t = blk * NG + g
                    if g != pf_g:
                        et = expp.tile([128, 4096], fp8, tag="exp")
                        nc.sync.dma_start(et[:], exp_d[t])
                    m0 = 128 * blk + 4 * g

                    if g in DVE_GROUPS:
                        # ---- DVE group ----
                        if g == pf_g:
                            etb = etb_pf
                        else:
                            etb = etbp.tile([128, 4096], bf16, tag="etb")
                            nc.scalar.activation(
                                etb[:], et[:],
                                mybir.ActivationFunctionType.Copy,
                            )
                        prod = prodp.tile([128, 4096], bf16, tag="prod")
                        in0 = etb[:].rearrange("p (mc k) -> p mc k", k=64)
                        in1 = (
                            adjA[:, 16 * m0 : 16 * m0 + 64]
                            .unsqueeze(2)
                            .broadcast_to([128, 64, 64])
                        )
                        nc.vector.tensor_tensor(
                            prod[:].rearrange("p (mc k) -> p mc k", k=64),
                            in0, in1, mybir.AluOpType.mult,
                        )
                        # tree over the 16 chunks: 16 -> 8 -> 4 -> 2
                        pv = prod[:].rearrange("p (a q) -> p a q", a=4)
                        nc.vector.tensor_add(
                            pv[:, :, 0:512], pv[:, :, 0:512], pv[:, :, 512:1024]
                        )
                        nc.vector.tensor_add(
                            pv[:, :, 0:256], pv[:, :, 0:256], pv[:, :, 256:512]
                        )
                        a4 = a4p.tile([128, 512], bf16, tag="a4")
                        a4v = a4[:].rearrange("p (a q) -> p a q", a=4)
                        nc.vector.tensor_add(
                            a4v[:, :, 0:128], pv[:, :, 0:128], pv[:, :, 128:256]
                        )
                        deferred.append((min(g + DEFER, 28), a4, g))
                    else:
                        # ---- PE group: 32 pair matmuls ----
                        ps = ps1p.tile([2, 512], f32, tag="ps1")
                        for mm in range(4):
                            m = m0 + mm
                            for q in range(8):
                                nc.tensor.matmul(
                                    ps[0:2, 128 * mm : 128 * (mm + 1)],
                                    adjA[:, 16 * m + 2 * q : 16 * m + 2 * q + 2],
                                    et[:, 1024 * mm + 128 * q : 1024 * mm + 128 * (q + 1)],
                                    start=(mm == 0 and q == 0),
                                    stop=(mm == 3 and q == 7),
                                )
                        nc.scalar.copy(
                            stage2[0:2, 512 * g : 512 * (g + 1)], ps[0:2, :]
                        )

                    # flush deferred DVE ones-matmuls
                    still = []
                    for g_at, a4, gsrc in deferred:
                        if g >= g_at:
                            pso = psop.tile([2, 512], f32, tag="pso")
                            nc.tensor.matmul(
                                pso[0:2, :], ones2[:, 0:2], a4[:, :],
                                start=True, stop=True,
                            )
                            nc.scalar.copy(
                                stage2[0:2, 512 * gsrc : 512 * (gsrc + 1)],
                                pso[0:2, :],
                            )
                        else:
                            still.append((g_at, a4, gsrc))
                    deferred = still

                    # G chunk matmuls (16 per block), evac on DVE
                    j = g - G_START
                    if 0 <= j < 16:
                        g2 = ps2p.tile([128, 512], f32, tag="ps2")
                        nc.tensor.matmul(
                            g2[:, :],
                            embT[:, 128 * blk : 128 * (blk + 1)],
                            wsb[:, 512 * j : 512 * (j + 1)],
                            start=True, stop=True,
                        )
                        if j % 2 == 0:
                            nc.scalar.copy(
                                gsb[:, 512 * j : 512 * (j + 1)], g2[:, :]
                            )
                        else:
                            nc.vector.tensor_copy(
                                gsb[:, 512 * j : 512 * (j + 1)], g2[:, :]
                            )

                    # interleave previous block's step-2 pieces
                    while pending_step2 and pending_step2[0][0] <= g:
                        _, fn = pending_step2.pop(0)
                        fn()

                for g_at, a4, gsrc in deferred:
                    pso = psop.tile([2, 512], f32, tag="pso")
                    nc.tensor.matmul(
                        pso[0:2, :], ones2[:, 0:2], a4[:, :], start=True, stop=True
                    )
                    nc.scalar.copy(
                        stage2[0:2, 512 * gsrc : 512 * (gsrc + 1)], pso[0:2, :]
                    )
                for _, fn in pending_step2:
                    fn()
                pending_step2 = []

                # two transpose DMAs: half-sums onto atom partitions
                aexp2 = aexp2p.tile([128, 128], f16, tag="aexp2")
                s0 = stage2[0:1, :].rearrange("p (a x) -> p a x", x=128)
                s1 = stage2[1:2, :].rearrange("p (a x) -> p a x", x=128)
                dq = nc.sync if blk == NB - 1 else nc.gpsimd
                dq.dma_start(aexp2[:, 0:64], s0[:, :, 0:64])
                dq.dma_start(aexp2[:, 64:128], s1[:, :, 64:128])
                aexp = aexpp.tile([128, K], f16, tag="aexp")
                nc.vector.tensor_add(aexp[:], aexp2[:, 0:64], aexp2[:, 64:128])

                pending_step2 = step2_pieces(blk, aexp, gsb, is_tail=(blk == NB - 1))

            for _, fn in pending_step2:
                fn()

    nc.compile()
    return nc


def _prep_inputs(dist_adj, dist_exp, atom_emb, bilinear_w, bilinear_b):
    dist_adj = np.asarray(dist_adj, dtype=np.float32)
    dist_exp = np.asarray(dist_exp, dtype=np.float32)
    atom_emb = np.asarray(atom_emb, dtype=np.float32)
    bilinear_w = np.asarray(bilinear_w, dtype=np.float32)
    bilinear_b = np.asarray(bilinear_b, dtype=np.float32)

    exp_b = (
        dist_exp.astype(_E3)
        .reshape(N_CORES, M // 4, 4, 128, 1024)
        .transpose(0, 1, 3, 2, 4)
        .reshape(N_CORES, M // 4, 128, 4096)
    )
    adjA = (
        dist_adj.reshape(N_CORES, M, 128, 16)
        .transpose(0, 2, 1, 3)
        .reshape(N_CORES, 128, 16 * M)
        .astype(_BF, order="C")
    )
    embT = atom_emb.reshape(N_CORES, M, H).transpose(0, 2, 1).astype(_BF, order="C")
    w2 = bilinear_w.transpose(1, 0, 2).reshape(H, K * OUT).astype(_BF, order="C")
    biasb = np.ascontiguousarray(
        np.broadcast_to(bilinear_b.astype(np.float32), (128, OUT))
    )
    onesb = np.ones((128, 2), dtype=_BF)

    in_maps = []
    for i in range(N_CORES):
        in_maps.append(
            {
                "exp": np.ascontiguousarray(exp_b[i]),
                "adjA": np.ascontiguousarray(adjA[i]),
                "embT": np.ascontiguousarray(embT[i]),
                "w": w2,
                "bias": biasb,
                "ones": onesb,
            }
        )
    return in_maps


def _run(in_maps, **kwargs):
    _ensure_path()
    from concourse.bass_utils import run_bass_kernel_spmd

    if "nc" not in _CACHE:
        _CACHE["nc"] = _build()
    nc = _CACHE["nc"]
    res = run_bass_kernel_spmd(nc, in_maps, core_ids=list(range(N_CORES)), **kwargs)
    return res


def kernel(dist_adj, dist_exp, atom_emb, bilinear_w, bilinear_b):
    in_maps = _prep_inputs(dist_adj, dist_exp, atom_emb, bilinear_w, bilinear_b)
    res = _run(in_maps)
    out = np.concatenate(
        [np.asarray(res.results[i]["out"]) for i in range(N_CORES)], axis=0
    )
    return out.astype(np.float32)


# revision 44
# speedup vs baseline: 1.1835x; 1.1835x over previous
# Trainium2 Bass kernel for AtomTypeGNN message passing.
#
#   adj_exp[m,k] = sum_n dist_adj[m,n] * dist_exp[m,n,k]          (streams dist_exp)
#   feat[m,o]    = sum_{f,h} adj_exp[m,f] * w[f,h,o] * emb[m,h]
#   out          = softplus(feat) + b
#
# Pure data parallel over atoms: 8 NeuronCores, 256 atoms each, no collectives.
#
# Final design, ~165us HW exec (baseline bf16 PE-only: ~290us):
#  * dist_exp streams as fp8 e3m4 (HW-measured l2 ~1.5e-2 vs the 2e-2 gate)
#    -- HBM bytes halve to ~33.6MB/core.
#  * The PE moving port (128 elem/cycle) cannot stream everything, so atom
#    groups split between consumers:
#    - PE groups: per atom, 8 accumulating matmuls with a [128,2] stationary
#      (two adjacent adj chunk-columns) and [128,128] moving (two chunks).
#      The [2,128] psum block accumulates even-chunk sums on row 0 cols 0:64
#      and odd-chunk sums on row 1 cols 64:128 (diagonal); off-diagonal is
#      junk. One [2,512] psum bank per 4-atom group, single start/stop.
#    - DVE groups: ScalarE casts the fp8 tile to bf16 (fp8-source DVE ops
#      run ~3-5x slow, bf16 runs near spec); DVE does ONE tensor_tensor per
#      group (adj broadcast over k via a stride-0 AP) and an in-place
#      pairwise add tree down to 2 partial sums per atom; a [128,2]
#      all-ones stationary matmul turns those into a [2,512] psum block
#      whose rows are identical, so cols 0:64/64:128 per atom mimic the PE
#      diag layout. Ones-matmuls are deferred several groups so the
#      in-order PE queue never waits on DVE.
#  * Uniform ScalarE evac [2,512]->fp16 stage2; per block two SBUF->SBUF
#    DMAs transpose the two half-sums onto atom partitions; one DVE add
#    forms adj_exp. No DRAM bounce.
#  * G_f = emb @ w[f]: 16 [128x512] chunk matmuls per block (embT
#    stationary), evacuated to bf16 gsb by DVE tensor_copy.
#  * step 2 per block: DVE STT chain over f<NCH + ScalarE scaled copies for
#    the rest, folded by a DVE pairwise tree; ScalarE-heavy mid-stream,
#    DVE-heavy in the tail. softplus = relu(x)+ln(1+exp(-min(|x|,87))).
#
# Queues: exp stream owns nc.sync (SP HWDGE); consts load on nc.scalar
# (Activation HWDGE); transpose + output DMAs ride gpsimd (SWDGE).

import numpy as np
import ml_dtypes

N = 2048
K = 64
H = 128
OUT = 128
N_CORES = 8
M = N // N_CORES  # 256 atoms per core

_BF = ml_dtypes.bfloat16
_E3 = ml_dtypes.float8_e3m4

_CACHE = {}

# tuning knobs
DVE_GROUPS = (2, 6, 10, 14, 18, 22)  # groups handled by DVE
DEFER = 12       # groups between a DVE group and its ones-matmul
G_START = 8      # first group index carrying a G chunk
NCH_STEADY = 40  # f-chain length on DVE for non-tail blocks
NCH_TAIL = 32    # f-chain length on DVE for the last block


def _ensure_path():
    import sys

    for p in ("/opt/trn_rl_repo",):
        if p not in sys.path:
            sys.path.insert(0, p)


def _build():
    _ensure_path()
    import concourse.bass as bass  # noqa: F401
    import concourse.tile as tile
    from concourse import bacc, mybir

    if not getattr(bacc, "_act_tbl_patched", False):
        _orig_gat = bacc.get_activation_tables

        def _gat(arch):
            t = _orig_gat(arch)
            key = "natural_log_exp_and_others"
            if key not in t:
                return t
            shared = t[key]
            out = {}
            for k, v in t.items():
                if k == key:
                    out[k] = v
                else:
                    out[k] = v - shared
            return out

        bacc.get_activation_tables = _gat
        bacc._act_tbl_patched = True


    f32 = mybir.dt.float32
    f16 = mybir.dt.float16
    bf16 = mybir.dt.bfloat16
    fp8 = mybir.dt.float8e3

    nc = bacc.Bacc(
        "TRN2",
        target_bir_lowering=False,
        debug=False,
        num_devices=N_CORES,
    )

    # [t, p, mq]: atom group t = atoms 4t..4t+3, partition p, mq = 1024*mm + q,
    # q = 64*c + k, n = 16p + c.  4 KiB contiguous per partition per group.
    exp_d = nc.declare_dram_parameter("exp", [M // 4, 128, 4096], fp8, isOutput=False)
    # adjA[j, 16m + c] = dist_adj[m, 16j + c]
    adjA_d = nc.declare_dram_parameter("adjA", [128, 16 * M], bf16, isOutput=False)
    embT_d = nc.declare_dram_parameter("embT", [H, M], bf16, isOutput=False)
    # w2[h, 128f + o] = bilinear_w[f, h, o]
    w_d = nc.declare_dram_parameter("w", [H, K * OUT], bf16, isOutput=False)
    bias_d = nc.declare_dram_parameter("bias", [128, OUT], f32, isOutput=False)
    ones_d = nc.declare_dram_parameter("ones", [128, 2], bf16, isOutput=False)
    out_d = nc.declare_dram_parameter("out", [M, OUT], f32, isOutput=True)

    NB = M // 128  # blocks of 128 atoms
    NG = 32        # groups per block

    with tile.TileContext(nc) as tc, nc.allow_low_precision(
        reason="fp16/bf16 adj_exp partials; fp8 quantization error dominates"
    ):
        with (
            tc.tile_pool(name="const", bufs=1) as constp,
            tc.tile_pool(name="exp", bufs=7) as expp,
            tc.tile_pool(name="etb", bufs=2) as etbp,
            tc.tile_pool(name="prod", bufs=2) as prodp,
            tc.tile_pool(name="a4", bufs=6) as a4p,
            tc.tile_pool(name="ps1", bufs=5, space="PSUM") as ps1p,
            tc.tile_pool(name="pso", bufs=1, space="PSUM") as psop,
            tc.tile_pool(name="ps2", bufs=2, space="PSUM") as ps2p,
            tc.tile_pool(name="stage", bufs=2) as stagep,
            tc.tile_pool(name="aexp2", bufs=2) as aexp2p,
            tc.tile_pool(name="aexp", bufs=2) as aexpp,
                        tc.tile_pool(name="gsb", bufs=2) as gsbp,
            tc.tile_pool(name="acc", bufs=3) as accp,
            tc.tile_pool(name="ybuf", bufs=4) as ybufp,
            tc.tile_pool(name="outp", bufs=4) as outp,
        ):
            adjA = constp.tile([128, 16 * M], bf16, tag="adjA")
            nc.scalar.dma_start(adjA[:], adjA_d[:, :])
            wsb = constp.tile([128, K * OUT], bf16, tag="wsb")
            nc.scalar.dma_start(wsb[:], w_d[:, :])
            embT = constp.tile([128, M], bf16, tag="embT")
            nc.scalar.dma_start(embT[:], embT_d[:, :])
            biassb = constp.tile([128, OUT], f32, tag="bias")
            nc.scalar.dma_start(biassb[:], bias_d[:, :])
            ones2 = constp.tile([128, 2], bf16, tag="ones")
            nc.scalar.dma_start(ones2[:], ones_d[:, :])

            # ---------------- step 2 emission helper ----------------
            def step2_pieces(blk, aexp, gsb, is_tail):
                # feat = sum_f aexp[:,f] * G_f  done in 4 f-quarters: one
                # broadcast tensor_tensor (gsb x aexp bcast over o) into a
                # fp16 ybuf, an in-place fp16 pairwise tree over the 16 f's,
                # then accumulate the quarter partial into a f32 acc. All on
                # DVE in few big ops; no per-f chain, no ScalarE copies.
                state = {}

                ybufs = {}

                def q_mult(q):
                    def emit():
                        # ScalarE materializes the aexp broadcast so the DVE
                        # multiply gets all-packed 2-byte operands (2x mode)
                        ar = ybufp.tile([128, 16 * OUT], f16, tag="ybuf")
                        ar_in = (
                            aexp[:, 16 * q : 16 * (q + 1)]
                            .unsqueeze(2)
                            .broadcast_to([128, 16, OUT])
                        )
                        if is_tail:
                            nc.scalar.activation(
                                ar[:].rearrange("p (f o) -> p f o", o=OUT),
                                ar_in,
                                mybir.ActivationFunctionType.Copy,
                            )
                        else:
                            nc.vector.tensor_copy(
                                ar[:].rearrange("p (f o) -> p f o", o=OUT),
                                ar_in,
                            )
                        yq = ybufp.tile([128, 16 * OUT], f16, tag="ybuf")
                        ybufs[q] = yq
                        nc.vector.tensor_tensor(
                            yq[:],
                            gsb[:, 2048 * q : 2048 * (q + 1)],
                            ar[:],
                            mybir.AluOpType.mult,
                        )
                    return emit

                def q_tree(qa, qb):
                    # interleave two quarters' pairwise trees so the DVE
                    # pipeline always has an independent op in flight
                    def emit():
                        w = 16
                        while w > 1:
                            h = w // 2
                            for q in (qa, qb):
                                yq = ybufs[q]
                                nc.vector.tensor_add(
                                    yq[:, 0 : h * OUT],
                                    yq[:, 0 : h * OUT],
                                    yq[:, h * OUT : 2 * h * OUT],
                                )
                            w = h
                        for q in (qa, qb):
                            yq = ybufs.pop(q)
                            acc = accp.tile([128, OUT], f32, tag="acc")
                            if state.get("acc") is None:
                                nc.vector.tensor_copy(acc[:], yq[:, 0:OUT])
                            else:
                                nc.vector.tensor_add(
                                    acc[:], state["acc"][:], yq[:, 0:OUT]
                                )
                            state["acc"] = acc
                    return emit

                def finish():
                    acc = state["acc"]
                    t_cl = outp.tile([128, OUT], f32, tag="outp")
                    nc.vector.tensor_scalar_min(t_cl[:], acc[:], 30.0)
                    t_exp = outp.tile([128, OUT], f32, tag="outp")
                    nc.scalar.activation(
                        t_exp[:], t_cl[:], mybir.ActivationFunctionType.Exp
                    )
                    t_ln = outp.tile([128, OUT], f32, tag="outp")
                    nc.scalar.activation(
                        t_ln[:], t_exp[:], mybir.ActivationFunctionType.Ln,
                        bias=1.0,
                    )
                    t_rel = outp.tile([128, OUT], f32, tag="outp")
                    nc.vector.tensor_scalar(
                        t_rel[:], acc[:], -30.0, 0.0,
                        mybir.AluOpType.add, mybir.AluOpType.max,
                    )
                    t_s = outp.tile([128, OUT], f32, tag="outp")
                    nc.vector.tensor_add(t_s[:], t_ln[:], t_rel[:])
                    ot = outp.tile([128, OUT], f32, tag="outp")
                    nc.vector.tensor_add(ot[:], t_s[:], biassb[:])
                    odq = nc.sync if is_tail else nc.gpsimd
                    odq.dma_start(
                        out_d[128 * blk : 128 * (blk + 1), :], ot[:]
                    )

                pieces = [
                    (0, q_mult(0)),
                    (2, q_mult(1)),
                    (4, q_tree(0, 1)),
                    (6, q_mult(2)),
                    (8, q_mult(3)),
                    (10, q_tree(2, 3)),
                    (13, finish),
                ]
                return pieces

            pending_step2 = []

            for blk in range(NB):
                gsb = gsbp.tile([128, K * OUT], bf16, tag="gsb")
                stage2 = stagep.tile([2, 128 * 128], f16, tag="stage")
                deferred = []

                for g in range(NG):
                    t = blk * NG + g
                    et = expp.tile([128, 4096], fp8, tag="exp")
                    nc.sync.dma_start(et[:], exp_d[t])
                    m0 = 128 * blk + 4 * g

                    if g in DVE_GROUPS:
                        # ---- DVE group ----
                        etb = etbp.tile([128, 4096], bf16, tag="etb")
                        nc.scalar.activation(
                            etb[:], et[:],
                            mybir.ActivationFunctionType.Copy,
                        )
                        prod = prodp.tile([128, 4096], bf16, tag="prod")
                        in0 = etb[:].rearrange("p (mc k) -> p mc k", k=64)
                        in1 = (
                            adjA[:, 16 * m0 : 16 * m0 + 64]
                            .unsqueeze(2)
                            .broadcast_to([128, 64, 64])
                        )
                        nc.vector.tensor_tensor(
                            prod[:].rearrange("p (mc k) -> p mc k", k=64),
                            in0, in1, mybir.AluOpType.mult,
                        )
                        # tree over the 16 chunks: 16 -> 8 -> 4 -> 2
                        pv = prod[:].rearrange("p (a q) -> p a q", a=4)
                        nc.vector.tensor_add(
                            pv[:, :, 0:512], pv[:, :, 0:512], pv[:, :, 512:1024]
                        )
                        nc.vector.tensor_add(
                            pv[:, :, 0:256], pv[:, :, 0:256], pv[:, :, 256:512]
                        )
                        a4 = a4p.tile([128, 512], bf16, tag="a4")
                        a4v = a4[:].rearrange("p (a q) -> p a q", a=4)
                        nc.vector.tensor_add(
                            a4v[:, :, 0:128], pv[:, :, 0:128], pv[:, :, 128:256]
                        )
                        deferred.append((min(g + DEFER, 28), a4, g))
                    else:
                        # ---- PE group: 32 pair matmuls ----
                        ps = ps1p.tile([2, 512], f32, tag="ps1")
                        for mm in range(4):
                            m = m0 + mm
                            for q in range(8):
                                nc.tensor.matmul(
                                    ps[0:2, 128 * mm : 128 * (mm + 1)],
                                    adjA[:, 16 * m + 2 * q : 16 * m + 2 * q + 2],
                                    et[:, 1024 * mm + 128 * q : 1024 * mm + 128 * (q + 1)],
                                    start=(mm == 0 and q == 0),
                                    stop=(mm == 3 and q == 7),
                                )
                        nc.scalar.copy(
                            stage2[0:2, 512 * g : 512 * (g + 1)], ps[0:2, :]
                        )

                    # flush deferred DVE ones-matmuls
                    still = []
                    for g_at, a4, gsrc in deferred:
                        if g >= g_at:
                            pso = psop.tile([2, 512], f32, tag="pso")
                            nc.tensor.matmul(
                                pso[0:2, :], ones2[:, 0:2], a4[:, :],
                                start=True, stop=True,
                            )
                            nc.scalar.copy(
                                stage2[0:2, 512 * gsrc : 512 * (gsrc + 1)],
                                pso[0:2, :],
                            )
                        else:
                            still.append((g_at, a4, gsrc))
                    deferred = still

                    # G chunk matmuls (16 per block), evac on DVE
                    j = g - G_START
                    if 0 <= j < 16:
                        g2 = ps2p.tile([128, 512], f32, tag="ps2")
                        nc.tensor.matmul(
                            g2[:, :],
                            embT[:, 128 * blk : 128 * (blk + 1)],
                            wsb[:, 512 * j : 512 * (j + 1)],
                            start=True, stop=True,
                        )
                        if j % 2 == 0:
                            nc.scalar.copy(
                                gsb[:, 512 * j : 512 * (j + 1)], g2[:, :]
                            )
                        else:
                            nc.vector.tensor_copy(
                                gsb[:, 512 * j : 512 * (j + 1)], g2[:, :]
                            )

                    # interleave previous block's step-2 pieces
                    while pending_step2 and pending_step2[0][0] <= g:
                        _, fn = pending_step2.pop(0)
                        fn()

                for g_at, a4, gsrc in deferred:
                    pso = psop.tile([2, 512], f32, tag="pso")
                    nc.tensor.matmul(
                        pso[0:2, :], ones2[:, 0:2], a4[:, :], start=True, stop=True
                    )
                    nc.scalar.copy(
                        stage2[0:2, 512 * gsrc : 512 * (gsrc + 1)], pso[0:2, :]
                    )
                for _, fn in pending_step2:
                    fn()
                pending_step2 = []

                # two transpose DMAs: half-sums onto atom partitions
                aexp2 = aexp2p.tile([128, 128], f16, tag="aexp2")
                s0 = stage2[0:1, :].rearrange("p (a x) -> p a x", x=128)
                s1 = stage2[1:2, :].rearrange("p (a x) -> p a x", x=128)
                dq = nc.sync if blk == NB - 1 else nc.gpsimd
                dq.dma_start(aexp2[:, 0:64], s0[:, :, 0:64])
                dq.dma_start(aexp2[:, 64:128], s1[:, :, 64:128])
                aexp = aexpp.tile([128, K], f16, tag="aexp")
                nc.vector.tensor_add(aexp[:], aexp2[:, 0:64], aexp2[:, 64:128])

                pending_step2 = step2_pieces(blk, aexp, gsb, is_tail=(blk == NB - 1))

            for _, fn in pending_step2:
                fn()

    nc.compile()
    return nc


def _prep_inputs(dist_adj, dist_exp, atom_emb, bilinear_w, bilinear_b):
    dist_adj = np.asarray(dist_adj, dtype=np.float32)
    dist_exp = np.asarray(dist_exp, dtype=np.float32)
    atom_emb = np.asarray(atom_emb, dtype=np.float32)
    bilinear_w = np.asarray(bilinear_w, dtype=np.float32)
    bilinear_b = np.asarray(bilinear_b, dtype=np.float32)

    exp_b = (
        dist_exp.astype(_E3)
        .reshape(N_CORES, M // 4, 4, 128, 1024)
        .transpose(0, 1, 3, 2, 4)
        .reshape(N_CORES, M // 4, 128, 4096)
    )
    adjA = (
        dist_adj.reshape(N_CORES, M, 128, 16)
        .transpose(0, 2, 1, 3)
        .reshape(N_CORES, 128, 16 * M)
        .astype(_BF, order="C")
    )
    embT = atom_emb.reshape(N_CORES, M, H).transpose(0, 2, 1).astype(_BF, order="C")
    w2 = bilinear_w.transpose(1, 0, 2).reshape(H, K * OUT).astype(_BF, order="C")
    biasb = np.ascontiguousarray(
        np.broadcast_to(bilinear_b.astype(np.float32), (128, OUT))
    )
    onesb = np.ones((128, 2), dtype=_BF)

    in_maps = []
    for i in range(N_CORES):
        in_maps.append(
            {
                "exp": np.ascontiguousarray(exp_b[i]),
                "adjA": np.ascontiguousarray(adjA[i]),
                "embT": np.ascontiguousarray(embT[i]),
                "w": w2,
                "bias": biasb,
                "ones": onesb,
            }
        )
    return in_maps


def _run(in_maps, **kwargs):
    _ensure_path()
    from concourse.bass_utils import run_bass_kernel_spmd

    if "nc" not in _CACHE:
        _CACHE["nc"] = _build()
    nc = _CACHE["nc"]
    res = run_bass_kernel_spmd(nc, in_maps, core_ids=list(range(N_CORES)), **kwargs)
    return res


def kernel(dist_adj, dist_exp, atom_emb, bilinear_w, bilinear_b):
    in_maps = _prep_inputs(dist_adj, dist_exp, atom_emb, bilinear_w, bilinear_b)
    res = _run(in_maps)
    out = np.concatenate(
        [np.asarray(res.results[i]["out"]) for i in range(N_CORES)], axis=0
    )
    return out.astype(np.float32)
